# revision 5
# baseline (speedup 1.0000x reference)
"""Trainium2 Bass kernel for nn_DownsampleBlock (FPS + KNN + linear/relu + maxpool).

Self-contained: hardcodes N=65536, M=16384, K=16, D=128.
kernel(**inputs) takes full unsharded inputs, runs the Bass kernel on the
axon-tunneled NeuronCores, returns (out [M,128] f32, pos[idx] [M,3] f32,
batch[idx] [M] i32).

FPS replicates the reference's fp32 arithmetic bitwise ((dx^2+dy^2)+dz^2,
running min, argmax first-index). KNN distances use the |p|^2 - 2qp expansion
on the TensorEngine (host-validated: output rel err ~1e-3 vs reference).
"""
import numpy as np

import concourse.bacc as bacc
import concourse.bass as bass
import concourse.mybir as mybir
from concourse.bass import ds
from concourse.bass_utils import run_bass_kernel_spmd
from concourse.masks import make_identity
from concourse.tile import TileContext

F32 = mybir.dt.float32
I32 = mybir.dt.int32
U32 = mybir.dt.uint32

N = 65536
M = N // 4
KNN = 16
D = 128
P, C = 128, 512          # FPS layout: point i -> (p = i // 512, c = i % 512)
NBLK = N // 512          # 128 point blocks
NTILE = M // 128         # 128 center tiles
NEG = -3.0e38


def _register_custom_ops():
    from concourse.dve_spec import (Spec, Src0, Src1, C0, C1, Idx, MaxNeg,
                                    lower, sq, minn, eq, select)
    from concourse.dve_spec import AluOp as SpecAluOp
    import concourse.dve_spec as dve_spec
    from concourse.dve_ops import DveOp, OPS
    from concourse.dve_uop import DveOpSpec
    import concourse.dve_ops as dve_ops_mod

    def mk(name, spec, subdim=False):
        for existing in OPS:
            if existing.name == name:
                return existing
        opcode = dve_ops_mod._CUSTOM_DVE_ROW_BASE + len(OPS)
        assert opcode < 0x20
        shas = {}
        for ver in ("v3", "v4"):
            r = DveOpSpec(name=name, opcode=opcode, uops=lower(spec, ver=ver),
                          rd1_en=dve_spec._has_src1(spec))
            shas[ver] = r.sha(ver)
        op = DveOp(name, spec, subdim=subdim, uops_sha=shas)
        OPS.append(op)
        dve_ops_mod._SUB_OPCODE_FOR_NAME[name] = opcode
        dve_ops_mod.CUSTOM_DVE_SPECS[name] = spec
        return op

    op_sq2 = mk("FPS_SQ2", Spec(
        body=sq(Src0 - C0) + sq(Src1 - C1),
        reference=lambda in0, in1, s0, s1, imm2: (
            ((in0 - s0) ** 2 + (in1 - s1) ** 2).astype(np.float32)),
    ))
    op_sq1p = mk("FPS_SQ1P", Spec(
        body=sq(Src0 - C0) + Src1,
        reference=lambda in0, in1, s0, s1, imm2: (
            ((in0 - s0) ** 2 + in1).astype(np.float32)),
    ))
    op_minacc = mk("FPS_MINACC", Spec(
        body=minn(Src0, Src1),
        accum=SpecAluOp.MAX,
        reference=lambda in0, in1, s0, s1, imm2: (
            np.minimum(in0, in1),
            np.minimum(in0, in1).max(axis=-1, keepdims=True)),
    ))

    def _pick_ref(in0, in1, s0, s1, imm2):
        n = in0.shape[-1]
        idx = np.arange(n, dtype=np.float32)
        o = np.where(idx[None, :] == s0, in0, -np.finfo(np.float32).max)
        return o.astype(np.float32), o.max(axis=-1, keepdims=True).astype(np.float32)

    op_pick = mk("FPS_PICK", Spec(
        body=select(eq(Idx, C0), Src0, MaxNeg),
        accum=SpecAluOp.MAX,
        reference=_pick_ref,
    ))
    return op_sq2, op_sq1p, op_minacc, op_pick


def _build(nc, n_steps=M - 1, fps_unroll=8):
    op_sq2, op_sq1p, op_minacc, op_pick = _register_custom_ops()

    x_d = nc.dram_tensor("x", [N, D], F32, kind="ExternalInput")
    pos_d = nc.dram_tensor("pos", [N, 3], F32, kind="ExternalInput")
    batch_d = nc.dram_tensor("batch", [N, 1], I32, kind="ExternalInput")
    w_d = nc.dram_tensor("w", [D, D], F32, kind="ExternalInput")
    b_d = nc.dram_tensor("b", [D, 1], F32, kind="ExternalInput")

    out_d = nc.dram_tensor("out", [M, D], F32, kind="ExternalOutput")
    qlT_d = nc.dram_tensor("qlT", [4, M], F32, kind="ExternalOutput")
    idx_d = nc.dram_tensor("idx", [1, M], I32, kind="ExternalOutput")
    bat_d = nc.dram_tensor("bout", [128, M // 128], I32, kind="ExternalOutput")
    h_d = nc.dram_tensor("h", [N, D], F32, kind="Internal")
    pt4_d = nc.dram_tensor("pt4", [4, N], F32, kind="Internal")

    with TileContext(nc) as tc:
        with tc.tile_pool(name="sb", bufs=1) as pool:
            # ------------- persistent tiles -------------
            pxyz = pool.tile([P, 3 * C], F32, tag="pxyz")
            pxyz3 = pxyz[:].rearrange("p (c k) -> p c k", k=3)
            px, py, pz = pxyz3[:, :, 0], pxyz3[:, :, 1], pxyz3[:, :, 2]
            min_d = pool.tile([P, C], F32, tag="min_d")
            t_tmp = pool.tile([P, C], F32, tag="t_tmp")
            u_tmp = pool.tile([P, C], F32, tag="u_tmp")
            Qs = pool.tile([P, 3], F32, tag="Qs")
            iota_f = pool.tile([P, 1], F32, tag="iota_f")
            ones_row = pool.tile([1, P], F32, tag="ones_row")
            ident = pool.tile([P, P], F32, tag="ident")
            qlistT = pool.tile([4, M], F32, tag="qlistT")   # rows x,y,z,-0.5
            idxlist = pool.tile([1, M], I32, tag="idxlist")
            wt = pool.tile([D, D], F32, tag="wt")           # W^T [in, out]
            bcol = pool.tile([D, 1], F32, tag="bcol")
            blkbase = pool.tile([P, 8 * NBLK], U32, tag="blkbase")

            # ------------- phase 0: consts + loads -------------
            pos_r = pos_d.ap().rearrange("(p c) k -> p (c k)", p=P)
            nc.sync.dma_start(pxyz[:], pos_r)
            nc.vector.memset(min_d[:], 1e30)
            make_identity(nc, ident[:])
            nc.vector.memset(ones_row[:], 1.0)
            iota_u = pool.tile([P, 1], U32, tag="iota_u")
            nc.gpsimd.iota(iota_u[:], [[0, 1]], base=0, channel_multiplier=1)
            nc.vector.tensor_copy(iota_f[:], iota_u[:])
            nc.vector.memset(qlistT[:], -0.5)
            nc.sync.dma_start(bcol[:], b_d.ap())
            nc.gpsimd.iota(blkbase[:], [[512, NBLK], [0, 8]], base=0,
                           channel_multiplier=0)

            with tc.tile_pool(name="ph0ps", bufs=1, space="PSUM") as psA:
                wtmp = pool.tile([D, D], F32, tag="wtmp")
                nc.sync.dma_start(wtmp[:], w_d.ap())
                wt_ps = psA.tile([D, D], F32, tag="wtps")
                nc.tensor.transpose(wt_ps[:], wtmp[:], ident[:])
                nc.vector.tensor_copy(wt[:], wt_ps[:])

            # ------------- phase 1: h = relu(x W^T + b) -> h_d -------------
            with (tc.tile_pool(name="ph1", bufs=3) as wk1,
                  tc.tile_pool(name="ph1ps", bufs=2, space="PSUM") as ps1):
                with tc.For_i(0, N // 128, 4) as bi:
                    for u in range(4):
                        xb = wk1.tile([128, D], F32, tag="xb")
                        nc.sync.dma_start(xb[:], x_d.ap()[ds((bi + u) * 128, 128), :])
                        xt_ps = ps1.tile([D, 128], F32, tag="xtps")
                        nc.tensor.transpose(xt_ps[:], xb[:], ident[:])
                        xt = wk1.tile([D, 128], F32, tag="xt")
                        nc.vector.tensor_copy(xt[:], xt_ps[:])
                        ht_ps = ps1.tile([D, 128], F32, tag="htps")
                        nc.tensor.matmul(ht_ps[:], wt[:], xt[:], start=True, stop=True)
                        ht = wk1.tile([D, 128], F32, tag="ht")
                        nc.scalar.activation(ht[:], ht_ps[:],
                                             mybir.ActivationFunctionType.Relu,
                                             bias=bcol[:], scale=1.0)
                        hb_ps = ps1.tile([128, D], F32, tag="hbps")
                        nc.tensor.transpose(hb_ps[:], ht[:], ident[:])
                        hb = wk1.tile([128, D], F32, tag="hb")
                        nc.vector.tensor_copy(hb[:], hb_ps[:])
                        nc.sync.dma_start(h_d.ap()[ds((bi + u) * 128, 128), :], hb[:])

            # ------------- phase 2: posT4 = [px,py,pz,|p|^2] -> pt4_d ------
            a_sq = pool.tile([P, C], F32, tag="asq")
            zcol = pool.tile([P, 1], F32, tag="zcol")
            nc.vector.memset(zcol[:], 0.0)
            nc.vector._custom_dve(op_sq2, out=t_tmp[:], in0=px, in1=py,
                                  s0=zcol[:], s1=zcol[:])
            nc.vector._custom_dve(op_sq1p, out=a_sq[:], in0=pz, in1=t_tmp[:],
                                  s0=zcol[:])
            pt4_r = pt4_d.ap().rearrange("k (p c) -> k p c", p=P)
            for half in (slice(0, 64), slice(64, 128)):
                for k in range(3):
                    nc.sync.dma_start(pt4_r[k][half], pxyz3[half, :, k])
                nc.sync.dma_start(pt4_r[3][half], a_sq[half, :])

            # ------------- phase 3: FPS -------------
            with (tc.tile_pool(name="sp", bufs=3) as spool,
                  tc.tile_pool(name="fpsps", bufs=1, space="PSUM") as psum):
                onehot0 = spool.tile([P, 1], F32, tag="onehotv")
                nc.vector.tensor_scalar(onehot0[:], iota_f[:], 0.0, scalar2=None,
                                        op0=mybir.AluOpType.is_equal)
                qrow_ps = psum.tile([1, 3], F32, tag="qrow")
                nc.tensor.matmul(qrow_ps[:], onehot0[:], pxyz[:, 0:3],
                                 start=True, stop=True)
                qrow0 = spool.tile([1, 3], F32, tag="qrow")
                nc.scalar.copy(qrow0[:], qrow_ps[:])
                qcol_ps = psum.tile([3, 1], F32, tag="qcol")
                nc.tensor.transpose(qcol_ps[:], qrow0[:], ident[0:1, 0:1])
                nc.scalar.copy(qlistT[0:3, 0:1], qcol_ps[:])
                Q_ps = psum.tile([P, 3], F32, tag="Qps")
                nc.tensor.matmul(Q_ps[:], ones_row[:], qrow0[:],
                                 start=True, stop=True)
                nc.scalar.copy(Qs[:], Q_ps[:])
                nc.vector.memset(idxlist[0:1, 0:1], 0)

                def fps_step(tval):
                    Mcol = spool.tile([P, 1], F32, tag="Mcol")
                    colsf = spool.tile([P, 1], F32, tag="colsf")
                    cols8 = spool.tile([P, 8], U32, tag="cols8")
                    packT = spool.tile([1, P], F32, tag="packT")
                    g8 = spool.tile([1, 8], F32, tag="g8")
                    p8 = spool.tile([1, 8], U32, tag="p8")
                    p8f = spool.tile([1, 1], F32, tag="p8f")
                    qrow = spool.tile([1, 3], F32, tag="qrow")
                    nc.vector._custom_dve(op_sq2, out=t_tmp[:], in0=px, in1=py,
                                          s0=Qs[:, 0:1], s1=Qs[:, 1:2])
                    nc.vector._custom_dve(op_sq1p, out=u_tmp[:], in0=pz,
                                          in1=t_tmp[:], s0=Qs[:, 2:3])
                    nc.vector._custom_dve(op_minacc, out=min_d[:], in0=min_d[:],
                                          in1=u_tmp[:], accum_out=Mcol[:])
                    nc.vector.max_index(out=cols8[:],
                                        in_max=Mcol[:].to_broadcast([P, 8]),
                                        in_values=min_d[:])
                    nc.vector.tensor_copy(colsf[:], cols8[:, 0:1])
                    ptA = psum.tile([1, P], F32, tag="ptA")
                    nc.tensor.transpose(ptA[:], Mcol[:], ident[:])
                    nc.scalar.copy(packT[:], ptA[:])
                    nc.vector.max(out=g8[:], in_=packT[:])
                    nc.vector.max_index(out=p8[:], in_max=g8[:], in_values=packT[:])
                    nc.vector.tensor_copy(p8f[:], p8[0:1, 0:1])
                    pb_ps = psum.tile([P, 1], F32, tag="pb")
                    nc.tensor.matmul(pb_ps[:], ones_row[:], p8f[:],
                                     start=True, stop=True)
                    onehotv = spool.tile([P, 1], F32, tag="onehotv")
                    nc.vector.tensor_tensor(onehotv[:], iota_f[:], pb_ps[:],
                                            mybir.AluOpType.is_equal)
                    linp = spool.tile([P, 1], F32, tag="linp")
                    nc.vector.scalar_tensor_tensor(out=linp[:], in0=iota_f[:],
                                                   scalar=512.0, in1=colsf[:],
                                                   op0=mybir.AluOpType.mult,
                                                   op1=mybir.AluOpType.add)
                    lin_ps = psum.tile([1, 1], F32, tag="linps")
                    nc.tensor.matmul(lin_ps[:], onehotv[:], linp[:],
                                     start=True, stop=True)
                    nc.scalar.copy(idxlist[0:1, ds(tval, 1)], lin_ps[:])
                    qcand = spool.tile([P, 3], F32, tag="qcand")
                    nc.vector._custom_dve(op_pick, out=t_tmp[:], in0=px,
                                          s0=colsf[:], accum_out=qcand[:, 0:1])
                    nc.vector._custom_dve(op_pick, out=t_tmp[:], in0=py,
                                          s0=colsf[:], accum_out=qcand[:, 1:2])
                    nc.vector._custom_dve(op_pick, out=t_tmp[:], in0=pz,
                                          s0=colsf[:], accum_out=qcand[:, 2:3])
                    qrow2 = psum.tile([1, 3], F32, tag="qrow2")
                    nc.tensor.matmul(qrow2[:], onehotv[:], qcand[:],
                                     start=True, stop=True)
                    nc.scalar.copy(qrow[:], qrow2[:])
                    qcol2 = psum.tile([3, 1], F32, tag="qcol")
                    nc.tensor.transpose(qcol2[:], qrow[:], ident[0:1, 0:1])
                    nc.scalar.copy(qlistT[0:3, ds(tval, 1)], qcol2[:])
                    Q2 = psum.tile([P, 3], F32, tag="Qps")
                    nc.tensor.matmul(Q2[:], ones_row[:], qrow[:],
                                     start=True, stop=True)
                    nc.scalar.copy(Qs[:], Q2[:])

                n_loop = (n_steps // fps_unroll) * fps_unroll
                if n_loop:
                    with tc.For_i(0, n_loop, fps_unroll) as iv:
                        for u in range(fps_unroll):
                            fps_step(iv + (u + 1))
                for s in range(n_loop, n_steps):
                    fps_step(s + 1)

                nc.sync.dma_start(qlT_d.ap(), qlistT[:])
                nc.sync.dma_start(idx_d.ap(), idxlist[:])

            # ------------- phase 3b: batch gather -------------
            with (tc.tile_pool(name="bg", bufs=3) as wkb,
                  tc.tile_pool(name="bgps", bufs=2, space="PSUM") as psb):
                with tc.For_i(0, M // 128, 1) as bj:
                    bif = wkb.tile([1, 128], F32, tag="bif")
                    nc.vector.tensor_copy(bif[:], idxlist[0:1, ds(bj * 128, 128)])
                    bi_ps = psb.tile([128, 1], F32, tag="bips")
                    nc.tensor.transpose(bi_ps[:], bif[:], ident[0:1, 0:1])
                    bidx = wkb.tile([128, 1], I32, tag="bidx")
                    nc.vector.tensor_copy(bidx[:], bi_ps[:])
                    bval = wkb.tile([128, 1], I32, tag="bval")
                    nc.gpsimd.indirect_dma_start(
                        out=bval[:], out_offset=None, in_=batch_d[:],
                        in_offset=bass.IndirectOffsetOnAxis(ap=bidx[:, 0:1], axis=0))
                    nc.sync.dma_start(bat_d.ap()[:, ds(bj, 1)], bval[:])

            # ------------- phase 4: KNN + gather + maxpool -------------
            with (tc.tile_pool(name="kb", bufs=1) as big,
                  tc.tile_pool(name="kw", bufs=2) as wk,
                  tc.tile_pool(name="kps", bufs=2, space="PSUM") as psk):
                with tc.For_i(0, NTILE, 1) as it:
                    lhs = wk.tile([4, 128], F32, tag="lhs")
                    nc.vector.tensor_scalar_mul(lhs[:],
                                                qlistT[:, ds(it * 128, 128)], 2.0)
                    candV = big.tile([P, 8 * NBLK], F32, tag="candV")
                    candI = big.tile([P, 8 * NBLK], U32, tag="candI")
                    with tc.For_i(0, NBLK, 4) as jb:
                        for u in range(4):
                            rhs = wk.tile([4, 512], F32, tag="rhs")
                            nc.sync.dma_start(
                                rhs[:], pt4_d.ap()[:, ds((jb + u) * 512, 512)])
                            s_ps = psk.tile([P, 512], F32, tag="sps")
                            nc.tensor.matmul(s_ps[:], lhs[:], rhs[:],
                                             start=True, stop=True)
                            sblk = wk.tile([P, 512], F32, tag="sblk")
                            nc.scalar.copy(sblk[:], s_ps[:])
                            v8 = wk.tile([P, 8], F32, tag="v8")
                            nc.vector.max(out=v8[:], in_=sblk[:])
                            i8 = wk.tile([P, 8], U32, tag="i8")
                            nc.vector.max_index(out=i8[:], in_max=v8[:],
                                                in_values=sblk[:])
                            nc.vector.tensor_copy(
                                candV[:, ds((jb + u) * 8, 8)], v8[:])
                            nc.vector.tensor_copy(
                                candI[:, ds((jb + u) * 8, 8)], i8[:])
                    # stage 2: top-16 of candidates
                    candIg = big.tile([P, 8 * NBLK], U32, tag="candIg")
                    nc.vector.tensor_tensor(candIg[:], candI[:], blkbase[:],
                                            mybir.AluOpType.add)
                    candIf = big.tile([P, 8 * NBLK], F32, tag="candIf")
                    nc.vector.tensor_copy(candIf[:], candIg[:])
                    r8a = wk.tile([P, 8], F32, tag="r8a")
                    nc.vector.max(out=r8a[:], in_=candV[:])
                    pos8a = wk.tile([P, 8], U32, tag="pos8a")
                    nc.vector.max_index(out=pos8a[:], in_max=r8a[:],
                                        in_values=candV[:])
                    candVz = big.tile([P, 8 * NBLK], F32, tag="candVz")
                    nc.vector.match_replace(out=candVz[:], in_to_replace=r8a[:],
                                            in_values=candV[:], imm_value=NEG)
                    r8b = wk.tile([P, 8], F32, tag="r8b")
                    nc.vector.max(out=r8b[:], in_=candVz[:])
                    pos8b = wk.tile([P, 8], U32, tag="pos8b")
                    nc.vector.max_index(out=pos8b[:], in_max=r8b[:],
                                        in_values=candVz[:])
                    posf = wk.tile([P, 16], F32, tag="posf")
                    nc.vector.tensor_copy(posf[:, 0:8], pos8a[:])
                    nc.vector.tensor_copy(posf[:, 8:16], pos8b[:])
                    idx16 = wk.tile([P, 16], F32, tag="idx16")
                    pscr = big.tile([P, 8 * NBLK], F32, tag="pscr")
                    for j in range(16):
                        nc.vector._custom_dve(op_pick, out=pscr[:], in0=candIf[:],
                                              s0=posf[:, j:j + 1],
                                              accum_out=idx16[:, j:j + 1])
                    idx16i = wk.tile([P, 16], I32, tag="idx16i")
                    nc.vector.tensor_copy(idx16i[:], idx16[:])
                    # gather h rows + elementwise max pool
                    acc = wk.tile([P, D], F32, tag="acc")
                    nc.gpsimd.indirect_dma_start(
                        out=acc[:], out_offset=None, in_=h_d[:],
                        in_offset=bass.IndirectOffsetOnAxis(ap=idx16i[:, 0:1],
                                                            axis=0))
                    for j in range(1, 16):
                        hjj = wk.tile([P, D], F32, tag=f"hj{j % 2}")
                        nc.gpsimd.indirect_dma_start(
                            out=hjj[:], out_offset=None, in_=h_d[:],
                            in_offset=bass.IndirectOffsetOnAxis(
                                ap=idx16i[:, j:j + 1], axis=0))
                        nc.vector.tensor_tensor(acc[:], acc[:], hjj[:],
                                                mybir.AluOpType.max)
                    nc.sync.dma_start(out_d.ap()[ds(it * 128, 128), :], acc[:])

    return nc


_CACHE = {}


def _get_nc(n_steps=M - 1):
    key = ("nc", n_steps)
    if key not in _CACHE:
        nc = bacc.Bacc("TRN2", target_bir_lowering=False, debug=False)
        _build(nc, n_steps=n_steps)
        nc.compile()
        _CACHE[key] = nc
    return _CACHE[key]


def kernel(x_Rd, pos_Rd, batch_Rd, W, b):
    nc = _get_nc()
    in_map = {
        "x": np.ascontiguousarray(x_Rd, dtype=np.float32),
        "pos": np.ascontiguousarray(pos_Rd, dtype=np.float32),
        "batch": np.ascontiguousarray(batch_Rd, dtype=np.int32).reshape(N, 1),
        "w": np.ascontiguousarray(W, dtype=np.float32),
        "b": np.ascontiguousarray(b, dtype=np.float32).reshape(D, 1),
    }
    res = run_bass_kernel_spmd(nc, [in_map] * 8, core_ids=list(range(8)))
    r = res.results[0]
    out = r["out"]
    pos_out = r["qlT"][0:3, :].T.copy()
    batch_out = r["bout"].T.reshape(M).astype(np.int32)
    return out, pos_out, batch_out
